# revision 1
# baseline (speedup 1.0000x reference)
"""Causal self-attention with RoPE on 8 Trainium2 NeuronCores.

Problem: B=4, T=2048, C=1024, 16 heads x 64 dim, fp32 reference.

Sharding: 8 cores = (batch b in 0..3) x (head-group g in 0..1, 8 heads each).
Each core computes qkv for its batch/head-slice (column-parallel qkv),
full attention for its 8 heads, and a row-parallel partial projection.
Host sums the two partial projections per batch (the "all-reduce").

Per-core kernel layout strategy:
  - Host pre-transposes x and weights so every matmul contraction dim is
    on SBUF partitions (fp32 DMA transpose is unsupported on-device).
  - Matmuls run in bf16 (4x faster than fp32 on the PE), fp32 PSUM accum.
  - qkv produced in [t, f] layout; RoPE applied along the free axis
    (fused with PSUM evacuation); q/k then PE-transposed to [d, t] in
    head-pair stacks (2 heads x 64 = 128 partitions).
  - Scores are computed TRANSPOSED: ST[tk, tq] = kT.T @ qT per head
    (two heads run concurrently in the PE array via row tiling).
  - exp on ScalarE straight out of PSUM (scale=1/8 folded in). No max
    subtraction: |scores|/8 < ~40 << 88, safe in fp32/bf16 range.
  - Causal masking: gpsimd affine_select zeroes the upper-triangular
    part of diagonal-straddling exp tiles.
  - attn@v: outT[d, tq] = v.T @ PT with a ones-column appended to v, so
    row 64 of the output accumulates the softmax denominator l for free.
  - Normalization: l broadcast across partitions with a K=1 matmul,
    fast reciprocal on DVE, multiply fused into the PSUM evacuation.
  - proj: row-parallel y_partial = outT.T @ wprojT, fp32 output.
"""

import sys
import threading

sys.path.insert(0, "/opt/trn_rl_repo")

import ml_dtypes
import numpy as np

import concourse.bass as bass
import concourse.mybir as mybir
from concourse import bacc
from concourse.bass_utils import run_bass_kernel_spmd
from concourse.masks import make_identity
from concourse.tile import TileContext

BF16 = ml_dtypes.bfloat16
F32 = mybir.dt.float32
BF = mybir.dt.bfloat16

B, T, C = 4, 2048, 1024
NH, D = 16, 64          # global heads
HL = 8                  # local heads per core
G = 2                   # head groups (cores per batch)
FL = 3 * HL * D         # 1536 local qkv rows
CL = HL * D             # 512 local out channels
P = 128
TQ = 512                # query-block width
NTT = T // P            # 16 t-tiles
NPAIR = HL // 2         # 4 head pairs


def build_nc():
    nc = bacc.Bacc("TRN2", target_bir_lowering=False, debug=False, num_devices=8)

    xT = nc.declare_dram_parameter("xT", [C, T], BF, isOutput=False)
    wqkvT = nc.declare_dram_parameter("wqkvT", [C, FL], BF, isOutput=False)
    wprojT = nc.declare_dram_parameter("wprojT", [CL, C], BF, isOutput=False)
    cos_t = nc.declare_dram_parameter("cos_t", [T, D // 2], F32, isOutput=False)
    msin_t = nc.declare_dram_parameter("msin_t", [T, D // 2], F32, isOutput=False)
    psin_t = nc.declare_dram_parameter("psin_t", [T, D // 2], F32, isOutput=False)
    y = nc.declare_dram_parameter("y", [T, C], F32, isOutput=True)

    Exp = mybir.ActivationFunctionType.Exp

    with TileContext(nc) as tc:
        with (
            tc.tile_pool(name="const", bufs=1) as const,
            tc.tile_pool(name="work", bufs=4) as work,
            tc.tile_pool(name="pt", bufs=6) as ptp,
            tc.tile_pool(name="small", bufs=6) as small,
            tc.tile_pool(name="psmm", bufs=2, space="PSUM") as psmm,
            tc.tile_pool(name="psst", bufs=4, space="PSUM") as psst,
            tc.tile_pool(name="psout", bufs=2, space="PSUM") as psout,
        ):
            # ---- persistent SBUF tensors ----
            XT = const.tile([P, C // P, T], BF, tag="XT")
            WQKV = const.tile([P, C // P, FL], BF, tag="WQKV")
            WPROJ = const.tile([P, CL // P, C], BF, tag="WPROJ")
            COS = const.tile([P, NTT, D // 2], F32, tag="COS")
            MSIN = const.tile([P, NTT, D // 2], F32, tag="MSIN")
            PSIN = const.tile([P, NTT, D // 2], F32, tag="PSIN")
            IDT = const.tile([P, P], BF, tag="IDT")
            ONES1 = const.tile([1, D], BF, tag="ONES1")
            V1 = const.tile([P, NTT, HL, D + 1], BF, tag="V1")
            QT = const.tile([P, NPAIR, T], BF, tag="QT")
            KT = const.tile([P, NPAIR, T], BF, tag="KT")
            ONORM = const.tile([P, NPAIR, T], BF, tag="ONORM")

            xTr = xT.rearrange("(ko p) t -> p ko t", p=P)
            wqr = wqkvT.rearrange("(ko p) f -> p ko f", p=P)
            for ko in range(C // P):  # per-slab loads so compute starts early
                nc.sync.dma_start(XT[:, ko, :], xTr[:, ko, :])
                nc.sync.dma_start(WQKV[:, ko, :], wqr[:, ko, :])
            nc.sync.dma_start(WPROJ[:], wprojT.rearrange("(ko p) o -> p ko o", p=P))
            nc.sync.dma_start(COS[:], cos_t.rearrange("(n p) d -> p n d", p=P))
            nc.sync.dma_start(MSIN[:], msin_t.rearrange("(n p) d -> p n d", p=P))
            nc.sync.dma_start(PSIN[:], psin_t.rearrange("(n p) d -> p n d", p=P))
            make_identity(nc, IDT[:])
            nc.gpsimd.memset(ONES1[:], 1.0)
            nc.gpsimd.memset(V1[:], 1.0)
            MASKS = const.tile([P, TQ // P, TQ], BF, tag="MASKS")
            nc.gpsimd.memset(MASKS[:], 1.0)
            for ml in range(TQ // P):  # keep where q - p - 128*ml >= 0
                nc.gpsimd.affine_select(
                    out=MASKS[:, ml, :], in_=MASKS[:, ml, :],
                    compare_op=mybir.AluOpType.is_ge, fill=0.0,
                    base=-P * ml, pattern=[[1, TQ]], channel_multiplier=-1)

            # ---- fused: qkv/RoPE/transpose interleaved with attention ----
            yr = y.rearrange("(n p) o -> p n o", p=P)
            for i in range(NTT):
                for j3 in range(3):  # 0:q 1:k 2:v
                    ps = psmm.tile([P, 512], F32, tag="mm")
                    for ko in range(C // P):
                        nc.tensor.matmul(
                            ps[:],
                            lhsT=XT[:, ko, i * P:(i + 1) * P],
                            rhs=WQKV[:, ko, j3 * 512:(j3 + 1) * 512],
                            start=(ko == 0),
                            stop=(ko == C // P - 1),
                        )
                    if j3 < 2:
                        ps4 = ps.rearrange("p (h e d) -> p h e d", h=HL, e=2)
                        cosb = COS[:, i, :].unsqueeze(1).unsqueeze(1).to_broadcast(
                            [P, HL, 2, D // 2])
                        msb = MSIN[:, i, :].unsqueeze(1).to_broadcast(
                            [P, HL, D // 2])
                        psb = PSIN[:, i, :].unsqueeze(1).to_broadcast(
                            [P, HL, D // 2])
                        a = work.tile([P, 512], BF, tag="ropeA")
                        a4 = a.rearrange("p (h e d) -> p h e d", h=HL, e=2)
                        b = work.tile([P, 512], BF, tag="ropeB")
                        b4 = b.rearrange("p (h e d) -> p h e d", h=HL, e=2)
                        nc.vector.tensor_mul(a4[:], ps4[:], cosb)
                        nc.vector.tensor_mul(b4[:, :, 0, :], ps4[:, :, 1, :], msb)
                        nc.vector.tensor_mul(b4[:, :, 1, :], ps4[:, :, 0, :], psb)
                        r = work.tile([P, 512], BF, tag="ropeR")
                        nc.gpsimd.tensor_add(r[:], a[:], b[:])
                        dst = QT if j3 == 0 else KT
                        for p4 in range(NPAIR):
                            tp = psst.tile([P, P], BF, tag="st")
                            nc.tensor.transpose(
                                tp[:], r[:, p4 * P:(p4 + 1) * P], IDT[:])
                            nc.vector.tensor_copy(
                                dst[:, p4, i * P:(i + 1) * P], tp[:])
                    else:
                        ps3 = ps.rearrange("p (h d) -> p h d", h=HL)
                        nc.vector.tensor_copy(V1[:, i, :, 0:D], ps3[:])


                if i % (TQ // P) == (TQ // P) - 1:
                    j = i // (TQ // P)
                    ntk = (TQ // P) * (j + 1)
                    qsl = slice(j * TQ, (j + 1) * TQ)
                    for p4 in range(NPAIR):
                        outA = psout.tile([D + 1, TQ], F32, tag="out")
                        outB = psout.tile([D + 1, TQ], F32, tag="out")
                        for m in range(ntk):
                            ksl = slice(m * P, (m + 1) * P)
                            stA = psst.tile([P, TQ], F32, tag="st")
                            stB = psst.tile([P, TQ], F32, tag="st")
                            nc.tensor.matmul(
                                stA[:], lhsT=KT[0:D, p4, ksl], rhs=QT[0:D, p4, qsl],
                                start=True, stop=True)
                            nc.tensor.matmul(
                                stB[:], lhsT=KT[D:P, p4, ksl], rhs=QT[D:P, p4, qsl],
                                start=True, stop=True, tile_position=(D, 0))
                            pA = ptp.tile([P, TQ], BF, tag="ptA")
                            pB = ptp.tile([P, TQ], BF, tag="ptB")
                            nc.scalar.activation(pA[:], stA[:], Exp, scale=0.125)
                            nc.scalar.activation(pB[:], stB[:], Exp, scale=0.125)
                            ml = m - (TQ // P) * j
                            if ml >= 0:  # diagonal-straddling tile: zero tk > tq
                                nc.gpsimd.affine_select(
                                    out=pA[:], in_=pA[:],
                                    compare_op=mybir.AluOpType.is_ge,
                                    fill=0.0, base=-P * ml,
                                    pattern=[[1, TQ]], channel_multiplier=-1)
                                nc.vector.tensor_mul(pB[:], pB[:], MASKS[:, ml, :])
                            nc.tensor.matmul(
                                outA[:], lhsT=V1[:, m, 2 * p4, :], rhs=pA[:],
                                start=(m == 0), stop=(m == ntk - 1))
                            nc.tensor.matmul(
                                outB[:], lhsT=V1[:, m, 2 * p4 + 1, :], rhs=pB[:],
                                start=(m == 0), stop=(m == ntk - 1))
                        for w, outp in ((0, outA), (1, outB)):
                            lrow = small.tile([1, TQ], F32, tag="lrow")
                            nc.vector.tensor_copy(lrow[:], outp[D:D + 1, :])
                            r_row = small.tile([1, TQ], F32, tag="rrow")
                            nc.vector.reciprocal_approx_fast(
                                out=r_row[:], in_=lrow[:])
                            r64 = small.tile([D, TQ], F32, tag="rsb")
                            nc.gpsimd.partition_broadcast(r64[:], r_row[:])
                            nc.vector.tensor_mul(
                                ONORM[w * D:(w + 1) * D, p4, qsl],
                                outp[0:D, :], r64[:])
                    # projection for the query block just finished
                    for i in range((TQ // P) * j, (TQ // P) * (j + 1)):
                        for n2 in range(C // 512):
                            ps = psmm.tile([P, 512], F32, tag="mm")
                            for kc in range(NPAIR):
                                nc.tensor.matmul(
                                    ps[:],
                                    lhsT=ONORM[:, kc, i * P:(i + 1) * P],
                                    rhs=WPROJ[:, kc, n2 * 512:(n2 + 1) * 512],
                                    start=(kc == 0),
                                    stop=(kc == NPAIR - 1),
                                )
                            ysb = work.tile([P, 512], F32, tag="ysb")
                            nc.scalar.copy(out=ysb[:], in_=ps[:])
                            nc.sync.dma_start(yr[:, i, n2 * 512:(n2 + 1) * 512], ysb[:])

    nc.compile()
    return nc


def prep_inputs(x, w_qkv, w_proj):
    """Build the 8 per-core input maps from the full-problem inputs."""
    x = np.asarray(x, dtype=np.float32)
    w_qkv = np.asarray(w_qkv, dtype=np.float32)
    w_proj = np.asarray(w_proj, dtype=np.float32)

    inv_freq = 1.0 / (10000.0 ** (np.arange(0, D, 2, dtype=np.float32) / D))
    tt = np.arange(T, dtype=np.float32)
    freqs = np.outer(tt, inv_freq).astype(np.float32)  # [T, 32]
    cos_t = np.cos(freqs).astype(np.float32)
    sin_t = np.sin(freqs).astype(np.float32)
    msin_t = (-sin_t).astype(np.float32)

    in_maps = []
    for core in range(8):
        b, g = divmod(core, G)
        sl = slice(g * CL, (g + 1) * CL)
        w_local = np.concatenate(
            [w_qkv[sl], w_qkv[C:][sl], w_qkv[2 * C:][sl]], axis=0)  # [1536, C]
        in_maps.append({
            "xT": np.ascontiguousarray(x[b].T).astype(BF16),
            "wqkvT": np.ascontiguousarray(w_local.T).astype(BF16),
            "wprojT": np.ascontiguousarray(w_proj[:, sl].T).astype(BF16),
            "cos_t": cos_t,
            "msin_t": msin_t,
            "psin_t": sin_t,
        })
    return in_maps


_NC_LOCK = threading.Lock()
_NC = None


def get_nc():
    global _NC
    with _NC_LOCK:
        if _NC is None:
            _NC = build_nc()
    return _NC


def run(nc, in_maps, **kw):
    res = run_bass_kernel_spmd(nc, in_maps, list(range(8)), **kw)
    parts = [res.results[c]["y"] for c in range(8)]
    out = np.stack([parts[2 * b] + parts[2 * b + 1] for b in range(B)])
    return out.astype(np.float32), res


def kernel(x, w_qkv, w_proj):
    out, _ = run(get_nc(), prep_inputs(x, w_qkv, w_proj))
    return out



# revision 7
# speedup vs baseline: 1.3717x; 1.3717x over previous
"""Causal self-attention with RoPE on 8 Trainium2 NeuronCores.

Problem: B=4, T=2048, C=1024, 16 heads x 64 dim, fp32 reference.

Sharding: 8 cores = (batch b in 0..3) x (head-group g in 0..1, 8 heads each).
Each core computes qkv for its batch/head-slice (column-parallel qkv),
full attention for its 8 heads, and a row-parallel partial projection.
Host sums the two partial projections per batch (the "all-reduce").

Per-core kernel layout strategy (v2 — software-pipelined):
  - q/k are produced DIRECTLY in transposed [d, t] layout by running the
    qkv matmul in [f, t] orientation (lhsT = W slab, rhs = xT slab), so
    no PE transposes are needed at all.
  - RoPE is applied in the transposed layout: rot_half becomes a
    32-partition block swap, done with two DVE multiplies against
    host-precomputed [d, t] cos/sin tables (sign folded into the sin
    table), an SBUF->SBUF DMA partition swap, and one DVE add.
  - v runs in the baseline [t, f] orientation straight into V1 (with a
    ones column appended so attn@v row 64 accumulates the softmax
    denominator for free).
  - Scores are computed TRANSPOSED: ST[tk, tq] = kT.T @ qT per head;
    the two heads of a pair run concurrently in the PE array via row
    tiling (tile_position), writing the two halves of ONE 2-bank PSUM
    tile [128, 1024].
  - exp runs on ScalarE as a single [128, 1024] ACTIVATE per (pair, m)
    straight out of PSUM (scale=1/8 folded in; no max subtraction:
    |scores|/8 << 88, safe in fp32/bf16 range).
  - Causal masking: ONE DVE multiply with a precomputed mask tile per
    diagonal-straddling [128, 1024] exp tile (both heads at once).
  - Normalization: softmax denominator row 64 -> fast reciprocal on
    DVE, gpsimd partition_broadcast, DVE multiply into ONORM.
  - proj: row-parallel y_partial = ONORM.T @ wprojT, fp32 output.
  - EMISSION IS SOFTWARE-PIPELINED: attention block j is interleaved
    with the qkv chains of block j+1 and the projection of block j-1,
    so the PE never starves (keeps the HAM clock gate at 2.4 GHz) and
    ScalarE exp overlaps PE matmul work throughout.
"""

import sys
import threading

sys.path.insert(0, "/opt/trn_rl_repo")

import ml_dtypes
import numpy as np

import concourse.bass as bass
import concourse.mybir as mybir
from concourse import bacc
from concourse.bass_utils import run_bass_kernel_spmd
from concourse.tile import TileContext

BF16 = ml_dtypes.bfloat16
F32 = mybir.dt.float32
BF = mybir.dt.bfloat16

B, T, C = 4, 2048, 1024
NH, D = 16, 64          # global heads
HL = 8                  # local heads per core
G = 2                   # head groups (cores per batch)
FL = 3 * HL * D         # 1536 local qkv rows
CL = HL * D             # 512 local out channels
P = 128
TQ = 512                # query-block width
NTT = T // P            # 16 t-tiles
NPAIR = HL // 2         # 4 head pairs
NB = T // TQ            # 4 query blocks
KO = C // P             # 8 contraction slabs


def build_nc():
    nc = bacc.Bacc("TRN2", target_bir_lowering=False, debug=False, num_devices=8)

    xT = nc.declare_dram_parameter("xT", [C, T], BF, isOutput=False)
    wqkvT = nc.declare_dram_parameter("wqkvT", [C, FL], BF, isOutput=False)
    wprojT = nc.declare_dram_parameter("wprojT", [CL, C], BF, isOutput=False)
    cosd = nc.declare_dram_parameter("cosd", [P, T], F32, isOutput=False)
    sinu = nc.declare_dram_parameter("sinu", [P, T], F32, isOutput=False)
    y = nc.declare_dram_parameter("y", [T, C], F32, isOutput=True)

    Exp = mybir.ActivationFunctionType.Exp

    with TileContext(nc) as tc:
        with (
            tc.tile_pool(name="const", bufs=1) as const,
            tc.tile_pool(name="rope", bufs=3) as rope,
            tc.tile_pool(name="pexp", bufs=4) as pexp,
            tc.tile_pool(name="yout", bufs=3) as yout,
            tc.tile_pool(name="nrm", bufs=2) as nrm,
            tc.tile_pool(name="psscore", bufs=2, space="PSUM") as psscore,
            tc.tile_pool(name="psout", bufs=1, space="PSUM") as psout,
            tc.tile_pool(name="psmm", bufs=2, space="PSUM") as psmm,
        ):
            # ---- persistent SBUF tensors ----
            XT = const.tile([P, KO, T], BF, tag="XT")
            WQKV = const.tile([P, KO, FL], BF, tag="WQKV")
            WPROJ = const.tile([P, CL // P, C], BF, tag="WPROJ")
            COS = const.tile([P, T], F32, tag="COS")
            SINU = const.tile([P, T], F32, tag="SINU")
            V1 = const.tile([P, NTT, HL, D + 1], BF, tag="V1")
            QT = const.tile([P, NPAIR, T], BF, tag="QT")
            KT = const.tile([P, NPAIR, T], BF, tag="KT")
            ONORM = const.tile([P, NPAIR, T], BF, tag="ONORM")
            MASKS = const.tile([P, TQ // P, 2 * TQ], BF, tag="MASKS")

            # ---- input DMAs (slab-interleaved so compute starts early) ----
            nc.sync.dma_start(COS[:], cosd[:, :])
            nc.sync.dma_start(SINU[:], sinu[:, :])
            xTr = xT.rearrange("(ko p) t -> p ko t", p=P)
            wqr = wqkvT.rearrange("(ko p) f -> p ko f", p=P)
            for ko in range(KO):
                nc.sync.dma_start(WQKV[:, ko, :], wqr[:, ko, :])
                nc.sync.dma_start(XT[:, ko, 0:TQ], xTr[:, ko, 0:TQ])
            for ko in range(KO):
                nc.sync.dma_start(XT[:, ko, TQ:T], xTr[:, ko, TQ:T])
            nc.sync.dma_start(WPROJ[:], wprojT.rearrange("(ko p) o -> p ko o", p=P))

            nc.gpsimd.memset(V1[:], 1.0)
            nc.gpsimd.memset(MASKS[:], 1.0)
            for ml in range(TQ // P):  # keep where tq - p - 128*ml >= 0
                for h in range(2):  # identical mask for both heads' halves
                    nc.gpsimd.affine_select(
                        out=MASKS[:, ml, h * TQ:(h + 1) * TQ],
                        in_=MASKS[:, ml, h * TQ:(h + 1) * TQ],
                        compare_op=mybir.AluOpType.is_ge, fill=0.0,
                        base=-P * ml, pattern=[[1, TQ]], channel_multiplier=-1)

            yr = y.rearrange("(n p) o -> p n o", p=P)

            # ---- emission helpers ----
            def emit_qk_chain(j, p4, qk):
                """Produce QT/KT pair p4 for t-block j, already transposed,
                with RoPE fused into the PSUM evacuation."""
                tsl = slice(j * TQ, (j + 1) * TQ)
                fb = qk * CL + p4 * P
                ps = psmm.tile([P, TQ], F32, tag="mm")
                for ko in range(KO):
                    nc.tensor.matmul(
                        ps[:], lhsT=WQKV[:, ko, fb:fb + P],
                        rhs=XT[:, ko, tsl],
                        start=(ko == 0), stop=(ko == KO - 1))
                a = rope.tile([P, TQ], BF, tag="ra")
                b = rope.tile([P, TQ], BF, tag="rb")
                bs = rope.tile([P, TQ], BF, tag="rs")
                nc.vector.tensor_mul(a[:], ps[:], COS[:, tsl])
                nc.vector.tensor_mul(b[:], ps[:], SINU[:, tsl])
                # rot_half = swap 32-partition blocks within each head
                for blk in range(4):
                    src = (blk ^ 1) * 32
                    nc.sync.dma_start(
                        bs[blk * 32:(blk + 1) * 32, :], b[src:src + 32, :])
                dst = QT if qk == 0 else KT
                nc.vector.tensor_add(dst[:, p4, tsl], a[:], bs[:])

            def emit_v_chain(i):
                ps = psmm.tile([P, TQ], F32, tag="mm")
                for ko in range(KO):
                    nc.tensor.matmul(
                        ps[:], lhsT=XT[:, ko, i * P:(i + 1) * P],
                        rhs=WQKV[:, ko, 2 * CL:3 * CL],
                        start=(ko == 0), stop=(ko == KO - 1))
                nc.vector.tensor_copy(
                    V1[:, i, :, 0:D], ps.rearrange("p (h d) -> p h d", h=HL))

            def emit_att_m(j, p4, m, ntk, outAB):
                qsl = slice(j * TQ, (j + 1) * TQ)
                ksl = slice(m * P, (m + 1) * P)
                st = psscore.tile([P, 2 * TQ], F32, tag="st")
                nc.tensor.matmul(
                    st[:, 0:TQ], lhsT=KT[0:D, p4, ksl], rhs=QT[0:D, p4, qsl],
                    start=True, stop=True)
                nc.tensor.matmul(
                    st[:, TQ:2 * TQ], lhsT=KT[D:P, p4, ksl],
                    rhs=QT[D:P, p4, qsl],
                    start=True, stop=True, tile_position=(D, 0))
                pt = pexp.tile([P, 2 * TQ], BF, tag="pt")
                nc.scalar.activation(pt[:], st[:], Exp, scale=0.125)
                ml = m - (TQ // P) * j
                if ml >= 0:  # diagonal-straddling tile: zero tk > tq
                    nc.vector.tensor_mul(pt[:], pt[:], MASKS[:, ml, :])
                nc.tensor.matmul(
                    outAB[0:D + 1, 0:TQ], lhsT=V1[:, m, 2 * p4, :],
                    rhs=pt[:, 0:TQ],
                    start=(m == 0), stop=(m == ntk - 1))
                nc.tensor.matmul(
                    outAB[0:D + 1, TQ:2 * TQ], lhsT=V1[:, m, 2 * p4 + 1, :],
                    rhs=pt[:, TQ:2 * TQ],
                    start=(m == 0), stop=(m == ntk - 1))

            def emit_norm(j, p4, outAB):
                qsl = slice(j * TQ, (j + 1) * TQ)
                l2 = nrm.tile([1, 2 * TQ], F32, tag="l2")
                nc.vector.tensor_copy(l2[:], outAB[D:D + 1, :])
                r2 = nrm.tile([1, 2 * TQ], F32, tag="r2")
                nc.vector.reciprocal_approx_fast(out=r2[:], in_=l2[:])
                for w in range(2):
                    r64 = nrm.tile([D, TQ], F32, tag="r64")
                    nc.gpsimd.partition_broadcast(
                        r64[:], r2[:, w * TQ:(w + 1) * TQ])
                    nc.vector.tensor_mul(
                        ONORM[w * D:(w + 1) * D, p4, qsl],
                        outAB[0:D, w * TQ:(w + 1) * TQ], r64[:])

            def emit_proj(i, n2):
                ps = psmm.tile([P, TQ], F32, tag="mm")
                for kc in range(NPAIR):
                    nc.tensor.matmul(
                        ps[:], lhsT=ONORM[:, kc, i * P:(i + 1) * P],
                        rhs=WPROJ[:, kc, n2 * TQ:(n2 + 1) * TQ],
                        start=(kc == 0), stop=(kc == NPAIR - 1))
                ysb = yout.tile([P, TQ], F32, tag="ysb")
                nc.vector.tensor_copy(ysb[:], ps[:])
                nc.sync.dma_start(yr[:, i, n2 * TQ:(n2 + 1) * TQ], ysb[:])

            def qkv_block_fillers(j):
                f = []
                for p4 in range(NPAIR):
                    for qk in range(2):
                        f.append(lambda j=j, p4=p4, qk=qk: emit_qk_chain(j, p4, qk))
                for s in range(TQ // P):
                    f.append(lambda i=j * (TQ // P) + s: emit_v_chain(i))
                return f

            def proj_block_fillers(j):
                return [lambda i=i, n2=n2: emit_proj(i, n2)
                        for i in range(j * (TQ // P), (j + 1) * (TQ // P))
                        for n2 in range(C // TQ)]

            # ---- software-pipelined main loop ----
            for fn in qkv_block_fillers(0):
                fn()
            for j in range(NB):
                fillers = []
                if j + 1 < NB:
                    fillers += qkv_block_fillers(j + 1)
                if j - 1 >= 0:
                    fillers += proj_block_fillers(j - 1)
                ntk = (TQ // P) * (j + 1)
                natt = NPAIR * ntk
                fi = 0
                k = 0
                for p4 in range(NPAIR):
                    outAB = psout.tile([D + 1, 2 * TQ], F32, tag="out")
                    for m in range(ntk):
                        emit_att_m(j, p4, m, ntk, outAB)
                        k += 1
                        while fi < len(fillers) and fi * natt < k * len(fillers):
                            fillers[fi]()
                            fi += 1
                    emit_norm(j, p4, outAB)
                while fi < len(fillers):
                    fillers[fi]()
                    fi += 1
            for fn in proj_block_fillers(NB - 1):
                fn()

    nc.compile()
    return nc


def prep_inputs(x, w_qkv, w_proj):
    """Build the 8 per-core input maps from the full-problem inputs."""
    x = np.asarray(x, dtype=np.float32)
    w_qkv = np.asarray(w_qkv, dtype=np.float32)
    w_proj = np.asarray(w_proj, dtype=np.float32)

    inv_freq = 1.0 / (10000.0 ** (np.arange(0, D, 2, dtype=np.float32) / D))
    tt = np.arange(T, dtype=np.float32)
    freqs = np.outer(tt, inv_freq).astype(np.float32)  # [T, 32]
    cos_t = np.cos(freqs).astype(np.float32)           # [T, 32]
    sin_t = np.sin(freqs).astype(np.float32)
    # [d, t] tables for the transposed layout, stacked for a head pair.
    # cosd[p, t] = cos(f[t, p%32]); sinu carries rot_half's sign:
    # +sin for p%64 in [0,32) (source for upper target), -sin for [32,64).
    cos64 = np.concatenate([cos_t.T, cos_t.T], axis=0)   # [64, T]
    sin64 = np.concatenate([sin_t.T, -sin_t.T], axis=0)  # [64, T]
    cosd = np.ascontiguousarray(
        np.concatenate([cos64, cos64], axis=0), dtype=np.float32)  # [128, T]
    sinu = np.ascontiguousarray(
        np.concatenate([sin64, sin64], axis=0), dtype=np.float32)

    in_maps = []
    for core in range(8):
        b, g = divmod(core, G)
        sl = slice(g * CL, (g + 1) * CL)
        w_local = np.concatenate(
            [w_qkv[sl], w_qkv[C:][sl], w_qkv[2 * C:][sl]], axis=0)  # [1536, C]
        in_maps.append({
            "xT": np.ascontiguousarray(x[b].T).astype(BF16),
            "wqkvT": np.ascontiguousarray(w_local.T).astype(BF16),
            "wprojT": np.ascontiguousarray(w_proj[:, sl].T).astype(BF16),
            "cosd": cosd,
            "sinu": sinu,
        })
    return in_maps


_NC_LOCK = threading.Lock()
_NC = None


def get_nc():
    global _NC
    with _NC_LOCK:
        if _NC is None:
            _NC = build_nc()
    return _NC


def run(nc, in_maps, **kw):
    res = run_bass_kernel_spmd(nc, in_maps, list(range(8)), **kw)
    parts = [res.results[c]["y"] for c in range(8)]
    out = np.stack([parts[2 * b] + parts[2 * b + 1] for b in range(B)])
    return out.astype(np.float32), res


def kernel(x, w_qkv, w_proj):
    out, _ = run(get_nc(), prep_inputs(x, w_qkv, w_proj))
    return out
